# revision 91
# baseline (speedup 1.0000x reference)
"""NodeFormerConv on 8 TRN2 cores (axon-tunneled).

The wall-clock of a call is dominated by the axon wire (host->device input
transfer at ~70-90 MB/s, output fetch at ~40-50 MB/s, ~80 ms RPC floor) and
by per-call jax.jit retrace/compile when going through
bass_utils.run_bass_kernel_spmd.  So the layout here is:

 * one cached jit(shard_map(bass_exec)) executable per edge-layout key --
   no retrace, no XLA/neuronx recompile, zero output buffers kept
   device-resident (not donated, so they are reusable),
 * wire-compressed inputs: z int8 (per-node scale), exp(gumbels) uint8
   (per-node-head scale), weights int8 (per-out-channel scale), one-hot
   edge columns uint8, edge row ids uint16, bias/sigmoid rows
   de-duplicated to [1,*] rows,
 * packed int8 output (64B payload + 4B f32 per-node scale per row),
   fetched shard-parallel and dequantized on host,
 * the device kernel is the same math as the f32 baseline with a small
   dequantize prologue (device compute is ~free next to the wire).
   End-to-end rel err ~1e-2 against the f32 reference (gate is 2e-2).

Sharding: node dim N=30000 -> 3750/core (padded 3840 = 30 chunks of 128).
Pass 1a: q/k/v projections, qp (local stab), dd_k stored (diag folded),
         local key-stab partials, v-table write.
Collectives: AllReduce-max key stab [1,4]; AllGather v-table [30000,256].
Pass 1b: kp=exp, KG=kp*g, kvs/ks_sum accumulation (PE, ones-column trick).
Collective: AllReduce-add kvs [260,300]; reshuffle to [30m, (d,k)+ks].
Pass 2:  z_num/z_den matmuls, divide+mean over K, edge conv via one-hot
         scatter matmul over indirect-gathered v rows, output projection.
"""

import math
from contextlib import ExitStack

import numpy as np
from ml_dtypes import bfloat16

import concourse.bass as bass
import concourse.tile as tile
from concourse import mybir, bacc, bass_isa
from concourse.masks import make_identity

F32 = mybir.dt.float32
BF16 = mybir.dt.bfloat16
I32 = mybir.dt.int32
U16 = mybir.dt.uint16
U8 = mybir.dt.uint8
I8 = mybir.dt.int8
I16 = mybir.dt.int16
AX = mybir.AxisListType
ALU = mybir.AluOpType
ACT = mybir.ActivationFunctionType

B, N, CIN, H, D, M, K, E = 1, 30000, 128, 4, 64, 30, 10, 480000
NCORE = 8
NSH = N // NCORE            # 3750
CH = 30                     # chunks per core
NPAD = CH * 128             # 3840
TAU = 0.25
EPS = 1e-6
ALPHA = (float(D) ** -0.25) * (TAU ** -0.5)   # folded into P
RATIO = float(M) ** -0.5
PADCOL = 200                # one-hot miss sentinel for pad edges (u8)
# device-gathered weight stream: wqkvT | woT | cst | brows (64B-aligned)
WOFF_WQKV, WOFF_WO, WOFF_CST, WOFF_BROWS = 0, 98304, 114688, 151552
WTOT = 155136
WSLICE = WTOT // NCORE      # 19392


# --------------------------------------------------------------- blob layout
def _layout(cwt):
    """Single packed u8 input blob per core: one wire transfer instead of 11
    (measured ~3ms per-arg staging overhead on the axon tunnel)."""
    entries = [
        # weights ride as one distinct 1/8 slice per core and are AllGathered
        # on device (they were the last 8x-replicated wire bytes)
        ("wslice", (1, WSLICE),    np.uint8,   U8),
        ("zscl",  (1, NSH),        bfloat16,   BF16),
        ("gscl",  (NSH, H),        np.int8,    I8),
        ("rsio",  (NSH, 2),        bfloat16,   BF16),
        ("erow",  (128, cwt),      np.uint16,  U16),
        ("zT",    (128, NSH),      np.int8,    I8),
        ("gexp",  (NSH, H * K),    np.uint8,   U8),
        # per-window exclusive/inclusive degree prefix tables (minus 1, for
        # the clamp01 step trick); replaces the expanded one-hot ecol table
        ("ecum",  (2 * CH, 128),   np.int16,   I16),
    ]
    lay, off = {}, 0
    for nm, shp, npdt, birdt in entries:
        nb = int(np.prod(shp)) * np.dtype(npdt).itemsize
        off = (off + 63) // 64 * 64
        lay[nm] = (off, nb, shp, npdt, birdt)
        off += nb
    totb = (off + 127) // 128 * 128
    return lay, totb


# ----------------------------------------------------------------- host prep
def _prep(z, edge_index, Wq_w, Wq_b, Wk_w, Wk_b, Wv_w, Wv_b, Wo_w, Wo_b, b,
          projection_matrix, gumbels):
    """Returns (arrs, cw, off, cwt): arrs = {"blob": [NCORE, totb] u8}.

    The edge-layout key (cwt) is derived first so the blob can be allocated
    up front; the z / gumbel / edge sections are then quantized by worker
    threads writing directly into views of the blob."""
    from concurrent.futures import ThreadPoolExecutor

    col32 = np.asarray(edge_index[1], np.int32)
    row32 = np.asarray(edge_index[0], np.int32)
    c_of0 = col32 // NSH
    local0 = col32 - c_of0 * NSH
    w_of0 = local0 // 128
    blk0 = (c_of0 * CH + w_of0).astype(np.uint8)
    ec = np.bincount(blk0, minlength=NCORE * CH).reshape(NCORE, CH)
    cw = [max(1, int(math.ceil(ec[:, w].max() / 128.0))) for w in range(CH)]
    off = np.cumsum([0] + cw)
    cwt = int(off[-1])

    lay, totb = _layout(cwt)
    blob = np.zeros((NCORE, totb), np.uint8)

    def sect(nm):
        boff, nb, shp, npdt, _ = lay[nm]
        return blob[:, boff:boff + nb].view(npdt).reshape((NCORE,) + shp)

    def work_z():
        z2 = np.asarray(z, np.float32).reshape(N, CIN)
        # int8 per-node quantization: round(z_n / s_n), s_n = max|z_n|/127
        s = np.maximum(np.abs(z2).max(axis=1), 1e-30) / 127.0    # [N]
        t = z2 * (1.0 / s)[:, None]                              # [N,128]
        np.rint(t, out=t)   # exact ints in [-127,127]; u8-view assign casts
        sect("zT")[:] = t.reshape(NCORE, NSH, CIN).transpose(0, 2, 1)
        sect("zscl")[:, 0, :] = s.reshape(NCORE, NSH).astype(bfloat16)

    def work_g():
        # exp(gumbels) quantized u8 with per-(node,head) power-of-2 scale
        # 2^e, e = ceil(log2(max/255)) shipped as int8; rint(g/2^e) <= 255
        # exactly since 2^e >= max/255
        g2 = np.exp(np.asarray(gumbels, np.float32)).reshape(N, H, K)
        gs = np.maximum(g2.max(axis=2), 1e-30)                   # [N,H]
        e = np.ceil(np.log2(gs / 255.0)).astype(np.float32)
        np.divide(g2, np.exp2(e)[..., None], out=g2)
        np.rint(g2, out=g2)
        sect("gexp")[:] = g2.reshape(NCORE, NSH, H * K)
        sect("gscl")[:] = e.reshape(NCORE, NSH, H)   # exact ints -> i8 cast

    def work_e():
        d_in = np.bincount(col32, minlength=N).astype(np.float32)
        d_out = np.bincount(row32, minlength=N).astype(np.float32)
        rsio = sect("rsio")
        rsio[:, :, 0] = (1.0 / np.sqrt(np.maximum(d_in, 1.0))
                         ).reshape(NCORE, NSH).astype(bfloat16)
        rsio[:, :, 1] = (1.0 / np.sqrt(np.maximum(d_out, 1.0))
                         ).reshape(NCORE, NSH).astype(bfloat16)
        # sort edges by column (u16 radix): groups by (core, window) AND
        # orders by column within each window, so slots form contiguous
        # per-column runs describable by degree prefix ranges
        order = np.argsort(col32.astype(np.uint16), kind="stable")
        rs, cs = row32[order], col32[order]
        c_of = cs // NSH
        local = cs - c_of * NSH
        w_of = local // 128
        blk = c_of * CH + w_of                  # sorted ascending
        starts = np.zeros(NCORE * CH, np.int64)
        np.cumsum(ec.reshape(-1)[:-1], out=starts[1:])
        slot = np.arange(E, dtype=np.int64) - starts[blk]
        off_arr = np.asarray(off[:-1], np.int64)
        pcol = off_arr[w_of] + slot // 128
        prow = slot % 128
        erow = sect("erow")
        # +row//NSH: v-table rows are gathered as 8 blocks of NSH+1 (row NSH
        # of each block carries stab partials), so global node id r lives at
        # gathered row r + r//NSH
        erow[c_of, prow, pcol] = (rs + rs // NSH).astype(np.uint16)
        # per-window degree prefix tables: cumlo-1 rows 0..CH-1, cumhi-1
        # rows CH..2CH-1; window w covers nodes [w*128, w*128+128) of the
        # core shard (zero-degree pad cols in the last window)
        degs = np.zeros((NCORE, CH * 128), np.int64)
        degs[:, :NSH] = d_in.astype(np.int64).reshape(NCORE, NSH)
        degs = degs.reshape(NCORE, CH, 128)
        cumhi = np.cumsum(degs, axis=2)
        cumlo = cumhi - degs
        ecum = sect("ecum").reshape(NCORE, 2, CH, 128)
        ecum[:, 0] = cumlo - 1
        ecum[:, 1] = cumhi - 1

    pool = ThreadPoolExecutor(3)
    futs = [pool.submit(w) for w in (work_z, work_g, work_e)]

    # weights quantized i8 with per-output-channel scale
    def _qw(w):
        w = np.asarray(w, np.float32)
        s = np.maximum(np.abs(w).max(axis=1), 1e-30) / 127.0    # [cout]
        q = np.rint(w / s[:, None]).astype(np.int8)
        return q.T, s.astype(np.float32)                        # [cin,cout], [cout]

    wqT, sq = _qw(Wq_w)
    wkT, sk = _qw(Wk_w)
    wvT, sv = _qw(Wv_w)
    woTq, so = _qw(Wo_w)
    wstream = np.zeros(WTOT, np.uint8)
    wstream[WOFF_WQKV:WOFF_WO].view(np.int8).reshape(3, 128, 256)[:] = \
        np.stack([wqT, wkT, wvT])
    wstream[WOFF_WO:WOFF_CST].view(np.int8).reshape(2, 128, 64)[:] = \
        np.stack([woTq[:128], woTq[128:]])

    # cst [128,72] f32: qkb(4) | pT2(60) | nh2(2) | q/k half scales(4) | pad(2)
    cst = np.zeros((128, 72), np.float32)
    cst[:, 0] = Wq_b[:128]
    cst[:, 1] = Wq_b[128:]
    cst[:, 2] = Wk_b[:128]
    cst[:, 3] = Wk_b[128:]
    pT = (ALPHA * np.asarray(projection_matrix, np.float32)).T  # [64,30]
    cst[0:64, 4:4 + M] = pT
    cst[64:128, 4 + M:4 + 2 * M] = pT
    cst[0:64, 64] = -0.5
    cst[64:128, 65] = -0.5
    cst[:, 66] = sq[:128]
    cst[:, 67] = sq[128:]
    cst[:, 68] = sk[:128]
    cst[:, 69] = sk[128:]
    wstream[WOFF_CST:WOFF_BROWS].view(np.float32).reshape(128, 72)[:] = cst

    # brows [1,896] f32: vb(256) | wob(64) | sig(256) | v scales(256) | wo scales(64)
    sig = 1.0 / (1.0 + np.exp(-np.asarray(b, np.float64)[0]))   # [H]
    brows = np.zeros((1, 896), np.float32)
    brows[0, 0:256] = np.asarray(Wv_b, np.float32)
    brows[0, 256:320] = np.asarray(Wo_b, np.float32)
    brows[0, 320:576] = np.repeat(sig.astype(np.float32), 64)
    brows[0, 576:832] = sv
    brows[0, 832:896] = so
    wstream[WOFF_BROWS:WTOT].view(np.float32).reshape(1, 896)[:] = brows
    sect("wslice")[:] = wstream.reshape(NCORE, 1, WSLICE)

    for f in futs:
        f.result()
    pool.shutdown(wait=False)
    return {"blob": blob}, cw, [int(x) for x in off], cwt


# ------------------------------------------------------------- device build
def _build(nc, tc, ctx, cw, off, cwt):
    lay, totb = _layout(cwt)
    blob = nc.dram_tensor("blob", [1, totb], U8, kind="ExternalInput").ap()
    io = {}
    for nm, (boff, nb, shp, npdt, birdt) in lay.items():
        seg = blob[0:1, boff:boff + nb].bitcast(birdt)
        if shp[0] == 1:
            io[nm] = seg
        else:
            io[nm] = seg.rearrange("o (r c) -> (o r) c", c=shp[1])
    # packed output: per node row = 64 bytes int8 payload + 2 bytes bf16 scale
    out_d = nc.dram_tensor("out", [NSH, 66], U8, kind="ExternalOutput").ap()

    dram = ctx.enter_context(tc.tile_pool(name="dram", bufs=1, space="DRAM"))
    # v-table in bf16: halves the [N,256] AllGather volume (the dominant
    # collective); costs ~0.2% on the conv term, well inside the 2e-2 gate.
    # Row NSH of each core's table carries its key-stab partials, fusing the
    # stab AllReduce-max into this AllGather (a common per-head stab constant
    # cancels in z_num/z_den, so its exact value only gates exp overflow).
    # v-table rows: 256 int8 + 2-byte bf16 per-row scale (258B) -- halves
    # the dominant AllGather again vs bf16 rows
    vtab_loc = dram.tile([NSH + 1, 258], U8)
    vtab_full = dram.tile([NCORE * (NSH + 1), 258], U8, addr_space="Shared")
    kvs_in = dram.tile([H * 65, 300], F32)
    kvs_out = dram.tile([H * 65, 300], F32, addr_space="Shared")
    wsl_in = dram.tile([1, WSLICE // 4], F32)
    wfull = dram.tile([1, WTOT // 4], F32, addr_space="Shared")

    const = ctx.enter_context(tc.tile_pool(name="const", bufs=1))
    big = ctx.enter_context(tc.tile_pool(name="big", bufs=1))

    # ---- decompress prologue: bf16/u8/u16 -> f32 working tiles
    with tc.tile_pool(name="stage", bufs=1) as stage:
        # gather the 8 per-core weight slices into the full weight stream
        wsl_sb = stage.tile([1, WSLICE], U8, name="wsl_sb")
        nc.sync.dma_start(wsl_sb[:], io["wslice"])
        nc.sync.dma_start(wsl_in[:], wsl_sb[:].bitcast(F32))
        nc.gpsimd.collective_compute(
            "AllGather", ALU.bypass, replica_groups=[list(range(NCORE))],
            ins=[wsl_in[:].opt()], outs=[wfull[:].opt()])
        wb = wfull[:].bitcast(U8)                      # [1, WTOT]
        io["wqkvT"] = wb[0:1, WOFF_WQKV:WOFF_WO].bitcast(I8).rearrange(
            "o (s r c) -> (o s) r c", s=3, r=128)
        io["woT"] = wb[0:1, WOFF_WO:WOFF_CST].bitcast(I8).rearrange(
            "o (s r c) -> (o s) r c", s=2, r=128)
        io["cst"] = wb[0:1, WOFF_CST:WOFF_BROWS].bitcast(F32).rearrange(
            "o (r c) -> (o r) c", c=72)
        io["brows"] = wb[0:1, WOFF_BROWS:WTOT].bitcast(F32)
        wq = const.tile([128, 256], F32)
        wk = const.tile([128, 256], F32)
        wv = const.tile([128, 256], F32)
        for wdst, idx in ((wq, 0), (wk, 1), (wv, 2)):
            wbf = stage.tile([128, 256], I8, name=f"wbf{idx}")
            nc.sync.dma_start(wbf[:], io["wqkvT"][idx])
            nc.vector.tensor_copy(wdst[:], wbf[:])
        woT0 = const.tile([128, 64], F32)
        woT1 = const.tile([128, 64], F32)
        for wdst, idx in ((woT0, 0), (woT1, 1)):
            wbf = stage.tile([128, 64], I8, name=f"obf{idx}")
            nc.sync.dma_start(wbf[:], io["woT"][idx])
            nc.vector.tensor_copy(wdst[:], wbf[:])
        cst = const.tile([128, 72], F32)
        nc.sync.dma_start(cst[:], io["cst"][:])
        brow_sb = stage.tile([1, 896], F32, name="brow_sb")
        nc.sync.dma_start(brow_sb[:], io["brows"][:])
        bb = const.tile([128, 896], F32)
        nc.gpsimd.partition_broadcast(bb[:], brow_sb[:], channels=128)
        zT = big.tile([128, NPAD], F32)
        zbf = stage.tile([128, NPAD], I8, name="zbf")
        nc.gpsimd.memset(zbf[:, NSH:NPAD], 0.0)
        nc.sync.dma_start(zbf[:, 0:NSH], io["zT"][:])
        nc.vector.tensor_copy(zT[:], zbf[:])
        zs_rb = stage.tile([1, NPAD], BF16, name="zs_rb")
        nc.gpsimd.memset(zs_rb[:, NSH:NPAD], 0.0)
        nc.sync.dma_start(zs_rb[:, 0:NSH], io["zscl"][:])
        zs_row = stage.tile([1, NPAD], F32, name="zs_row")
        nc.vector.tensor_copy(zs_row[:], zs_rb[:])
        zs_b = stage.tile([128, NPAD], F32, name="zs_b")
        nc.gpsimd.partition_broadcast(zs_b[:], zs_row[:], channels=128)
        nc.vector.tensor_tensor(zT[:], zT[:], zs_b[:], op=ALU.mult)
    qkb = cst[:, 0:4]
    pT2 = cst[:, 4:64]
    nh2 = cst[:, 64:66]
    wsqk = cst[:, 66:70]          # per-partition q/k out-channel scales (halves)
    vb = bb[:, 0:256]
    wob = bb[:, 256:320]
    sigb = bb[:, 320:576]
    vscl = bb[:, 576:832]
    woscl = bb[:, 832:896]

    ident = const.tile([128, 128], F32)
    make_identity(nc, ident[:])
    iota_i = const.tile([128, 128], I32)
    nc.gpsimd.iota(iota_i[:], pattern=[[1, 128]], base=0, channel_multiplier=0)
    iota_f = const.tile([128, 128], F32)
    nc.vector.tensor_copy(iota_f[:], iota_i[:])
    with tc.tile_pool(name="ps_it", bufs=1, space="PSUM") as ps_it:
        itp = ps_it.tile([128, 128], F32)
        nc.tensor.transpose(itp[:], iota_f[:], ident[:])
        iota_pf = const.tile([128, 1], F32)     # [p,0] = p
        nc.vector.tensor_copy(iota_pf[:], itp[:, 0:1])

    qpT_h = [big.tile([30, NPAD], F32, name=f"qpT{h}") for h in range(H)]
    dd_all = big.tile([128, H * M * CH], F32)       # col = h*900 + c*30
    v_all = big.tile([128, CH * 260], F32)          # per chunk [65*4]
    stabpart = big.tile([128, 4 * CH], F32)         # col = c*4 + (2*half+hh)
    nc.gpsimd.memset(stabpart[:], -1e30)
    kvs_rhs_h = [big.tile([30, 650], F32, name=f"kvsr{h}") for h in range(H)]

    # ---------------- pass 1a ----------------
    with tc.tile_pool(name="p1a", bufs=3) as wk1, \
         tc.tile_pool(name="ps_qkv", bufs=2, space="PSUM") as ps_qkv, \
         tc.tile_pool(name="ps_sm", bufs=1, space="PSUM") as ps_sm:
        for c in range(CH):
            rows = NSH - c * 128 if c == CH - 1 else 128
            zsl = zT[:, c * 128:(c + 1) * 128]
            for qi, (wmat, bcol0) in enumerate([(wq, 0), (wk, 2)]):
                for hf in range(2):
                    qps = ps_qkv.tile([128, 128], F32, name="qps")
                    nc.tensor.matmul(qps[:], lhsT=wmat[:, hf * 128:(hf + 1) * 128],
                                     rhs=zsl, start=True, stop=True)
                    qsb = wk1.tile([128, 128], F32, name="qsb")
                    nc.vector.tensor_scalar(
                        qsb[:], qps[:], wsqk[:, bcol0 + hf:bcol0 + hf + 1],
                        qkb[:, bcol0 + hf:bcol0 + hf + 1],
                        op0=ALU.mult, op1=ALU.add)
                    sq = wk1.tile([128, 128], F32, name="sq")
                    nc.scalar.activation(sq[:], qsb[:], ACT.Square, scale=ALPHA)
                    dg = ps_sm.tile([128, 2], F32, name="dg")
                    nc.tensor.matmul(dg[:], lhsT=sq[:], rhs=nh2[:],
                                     start=True, stop=True)
                    dd = ps_sm.tile([128, 60], F32, name="dd")
                    nc.tensor.matmul(dd[:], lhsT=qsb[:], rhs=pT2[:],
                                     start=True, stop=True)
                    smax = wk1.tile([128, 2], F32, name="smax")
                    nc.vector.tensor_reduce(
                        smax[:], dd[:].rearrange("p (h m) -> p h m", h=2),
                        axis=AX.X, op=ALU.max)
                    if qi == 0:  # ---- query: exp with local stab
                        bias2 = wk1.tile([128, 2], F32, name="bias2")
                        nc.vector.tensor_tensor(bias2[:], dg[:], smax[:],
                                                op=ALU.subtract)
                        qp2 = wk1.tile([128, 60], F32, name="qp2")
                        for hh in range(2):
                            nc.scalar.activation(
                                qp2[:, hh * 30:(hh + 1) * 30],
                                dd[:, hh * 30:(hh + 1) * 30], ACT.Exp,
                                bias=bias2[:, hh:hh + 1])
                        nc.vector.tensor_scalar(qp2[:], qp2[:], EPS, RATIO,
                                                op0=ALU.add, op1=ALU.mult)
                        for hh in range(2):
                            tpq = ps_sm.tile([30, 128], F32, name="tpq")
                            nc.tensor.transpose(
                                tpq[:], qp2[:, hh * 30:(hh + 1) * 30],
                                ident[:])
                            nc.vector.tensor_copy(
                                qpT_h[hf * 2 + hh][:, c * 128:(c + 1) * 128],
                                tpq[:])
                    else:  # ---- key: store stab partials + dd' (diag folded)
                        nc.vector.tensor_copy(
                            stabpart[0:rows, c * 4 + hf * 2:c * 4 + hf * 2 + 2],
                            smax[0:rows, :])
                        dgs = wk1.tile([128, 2], F32, name="dgs")
                        nc.vector.tensor_copy(dgs[:], dg[:])
                        for hh in range(2):
                            h = hf * 2 + hh
                            nc.scalar.activation(
                                dd_all[:, h * (M * CH) + c * M:
                                       h * (M * CH) + (c + 1) * M],
                                dd[:, hh * 30:(hh + 1) * 30], ACT.Identity,
                                bias=dgs[:, hh:hh + 1])
            # ---- v (node-major)
            vps = ps_qkv.tile([128, 256], F32, name="vps")
            nc.tensor.matmul(vps[:], lhsT=zsl, rhs=wv[:], start=True, stop=True)
            vsb = wk1.tile([128, 256], F32, name="vsb")
            nc.vector.tensor_tensor(vsb[:], vps[:], vscl, op=ALU.mult)
            nc.vector.tensor_add(vsb[:], vsb[:], vb)
            nc.gpsimd.memset(v_all[:, c * 260:(c + 1) * 260], 1.0)
            for h in range(H):
                nc.vector.tensor_copy(
                    v_all[:, c * 260 + h * 65:c * 260 + h * 65 + 64],
                    vsb[:, h * 64:(h + 1) * 64])
            rsob = wk1.tile([128, 1], BF16, name="rsob")
            if rows < 128:
                nc.gpsimd.memset(rsob[:], 0.0)
            nc.sync.dma_start(rsob[0:rows, :],
                              io["rsio"][c * 128:c * 128 + rows, 1:2])
            rso = wk1.tile([128, 1], F32, name="rso")
            nc.vector.tensor_copy(rso[:], rsob[:])
            vsc = wk1.tile([128, 256], F32, name="vsc")
            nc.vector.tensor_scalar(vsc[:], vsb[:], rso[:, 0:1], None,
                                    op0=ALU.mult)
            vab = wk1.tile([128, 256], F32, name="vab")
            nc.scalar.activation(vab[:], vsc[:], ACT.Abs)
            vam = wk1.tile([128, 1], F32, name="vam")
            nc.vector.tensor_reduce(vam[:], vab[:], axis=AX.X, op=ALU.max)
            nc.vector.tensor_scalar(vam[:], vam[:], 1e-30, None, op0=ALU.add)
            vamb = wk1.tile([128, 1], BF16, name="vamb")
            nc.vector.tensor_copy(vamb[:], vam[:])
            vam2 = wk1.tile([128, 1], F32, name="vam2")
            nc.vector.tensor_copy(vam2[:], vamb[:])
            vrec = wk1.tile([128, 1], F32, name="vrec")
            nc.vector.reciprocal(vrec[:], vam2[:])
            nc.vector.tensor_scalar(vrec[:], vrec[:], 127.0, None,
                                    op0=ALU.mult)
            vqf = wk1.tile([128, 256], F32, name="vqf")
            nc.vector.tensor_scalar(vqf[:], vsc[:], vrec[:, 0:1], None,
                                    op0=ALU.mult)
            vqi = wk1.tile([128, 256], I8, name="vqi")
            nc.vector.tensor_copy(vqi[:], vqf[:])
            nc.sync.dma_start(
                vtab_loc[c * 128:c * 128 + rows, 0:256].bitcast(I8),
                vqi[0:rows, :])
            nc.sync.dma_start(
                vtab_loc[c * 128:c * 128 + rows, 256:258].bitcast(BF16),
                vamb[0:rows, :])

    # ------- v-table all-gather (stab partials ride along as row NSH) -----
    with tc.tile_pool(name="stb", bufs=1) as stb:
        stab4 = stb.tile([128, 4], F32)
        nc.vector.tensor_reduce(
            stab4[:], stabpart[:].rearrange("p (c h) -> p h c", h=4),
            axis=AX.X, op=ALU.max)
        stab4r = stb.tile([128, 4], F32)
        nc.gpsimd.partition_all_reduce(stab4r[:], stab4[:], channels=128,
                                       reduce_op=bass_isa.ReduceOp.max)
        stab4b = stb.tile([1, 4], BF16)
        nc.vector.tensor_copy(stab4b[:], stab4r[0:1, :])
        nc.sync.dma_start(vtab_loc[NSH:NSH + 1, 0:8].bitcast(BF16),
                          stab4b[:])
        nc.gpsimd.collective_compute(
            "AllGather", ALU.bypass, replica_groups=[list(range(NCORE))],
            ins=[vtab_loc[:].opt()], outs=[vtab_full[:].opt()])
        # extract the 8 gathered stab rows, reduce max across cores
        srows = vtab_full[:].rearrange("(c r) d -> c r d",
                                       r=NSH + 1)[:, NSH, 0:8].bitcast(BF16)
        stab8b = stb.tile([NCORE, 4], BF16)
        nc.sync.dma_start(stab8b[:], srows)
        stab8 = stb.tile([NCORE, 4], F32)
        nc.vector.tensor_copy(stab8[:], stab8b[:])
        stab8r = stb.tile([NCORE, 4], F32)
        nc.gpsimd.partition_all_reduce(stab8r[:], stab8[:], channels=NCORE,
                                       reduce_op=bass_isa.ReduceOp.max)
        stab_b = big.tile([128, 4], F32)
        nc.gpsimd.partition_broadcast(stab_b[:], stab8r[0:1, :], channels=128)
        negstab = big.tile([128, 4], F32)
        nc.vector.tensor_scalar(negstab[:], stab_b[:], -1.0, None, op0=ALU.mult)

    # ---------------- pass 1b: kvs accumulation ----------------
    with tc.tile_pool(name="p1b", bufs=3) as wk2, \
         tc.tile_pool(name="ps_kvs", bufs=1, space="PSUM") as ps_kvs:
        kvsp = [ps_kvs.tile([65, 300], F32, name=f"kvsp{h}") for h in range(H)]
        for c in range(CH):
            grows = 128 if c < CH - 1 else NSH - (CH - 1) * 128
            gt = wk2.tile([128, 40], U8, name="gt")
            gsb = wk2.tile([128, 4], I8, name="gsb")
            if grows < 128:
                nc.gpsimd.memset(gt[:], 0.0)
                nc.gpsimd.memset(gsb[:], 0.0)
            nc.sync.dma_start(gt[0:grows, :],
                              io["gexp"][c * 128:c * 128 + grows, :])
            nc.sync.dma_start(gsb[0:grows, :],
                              io["gscl"][c * 128:c * 128 + grows, :])
            gsf = wk2.tile([128, 4], F32, name="gsf")
            nc.vector.tensor_copy(gsf[:], gsb[:])
            # decode power-of-2 scale: 2^e = exp(e * ln2)
            nc.scalar.activation(gsf[:], gsf[:], ACT.Exp,
                                 scale=0.6931471805599453)
            ge = wk2.tile([128, 40], F32, name="ge")
            nc.vector.tensor_copy(ge[:], gt[:])
            nc.vector.tensor_tensor(
                ge[:].rearrange("p (h k) -> p h k", h=4),
                ge[:].rearrange("p (h k) -> p h k", h=4),
                gsf[:].rearrange("p (h o) -> p h o", o=1)
                      .to_broadcast([128, 4, 10]),
                op=ALU.mult)
            kp2 = wk2.tile([128, 120], F32, name="kp2")
            for h in range(H):
                nc.scalar.activation(
                    kp2[:, h * 30:(h + 1) * 30],
                    dd_all[:, h * (M * CH) + c * M:h * (M * CH) + (c + 1) * M],
                    ACT.Exp, bias=negstab[:, h:h + 1])
            nc.vector.tensor_scalar(kp2[:], kp2[:], EPS, RATIO,
                                    op0=ALU.add, op1=ALU.mult)
            for h in range(H):
                kg = wk2.tile([128, 300], F32, name="kg")
                nc.vector.tensor_tensor(
                    kg[:].rearrange("p (k m) -> p k m", k=10),
                    kp2[:, h * 30:(h + 1) * 30]
                        .rearrange("p (o m) -> p o m", o=1)
                        .to_broadcast([128, 10, 30]),
                    ge[:, h * 10:(h + 1) * 10]
                        .rearrange("p (k o) -> p k o", o=1)
                        .to_broadcast([128, 10, 30]),
                    op=ALU.mult)
                nc.tensor.matmul(
                    kvsp[h][:], lhsT=v_all[:, c * 260 + h * 65:c * 260 + (h + 1) * 65],
                    rhs=kg[:], start=(c == 0), stop=(c == CH - 1))
        for h in range(H):
            ksb = wk2.tile([65, 300], F32, name="ksb")
            nc.vector.tensor_copy(ksb[:], kvsp[h][:])
            nc.sync.dma_start(kvs_in[h * 65:(h + 1) * 65, :], ksb[:])

    nc.gpsimd.collective_compute(
        "AllReduce", ALU.add, replica_groups=[list(range(NCORE))],
        ins=[kvs_in[:].opt()], outs=[kvs_out[:].opt()])

    # ---------------- kvs reshuffle: [65,(k,m)] -> [30m, (d,k)|ks] --------
    with tc.tile_pool(name="rsh", bufs=2) as rsh, \
         tc.tile_pool(name="ps_rsh", bufs=1, space="PSUM") as ps_rsh:
        for h in range(H):
            kar = rsh.tile([65, 300], F32, name="kar")
            nc.sync.dma_start(kar[:], kvs_out[h * 65:(h + 1) * 65, :])
            for kk in range(K):
                tp = ps_rsh.tile([30, 65], F32, name="tp")
                nc.tensor.transpose(tp[:], kar[:, kk * 30:(kk + 1) * 30],
                                    ident[0:65, 0:65])
                nc.vector.tensor_copy(
                    kvs_rhs_h[h][:, :640]
                        .rearrange("p (d k) -> p d k", k=10)[:, :, kk:kk + 1],
                    tp[:, 0:64].rearrange("p (d o) -> p d o", o=1))
                nc.vector.tensor_copy(
                    kvs_rhs_h[h][:, 640 + kk:641 + kk], tp[:, 64:65])

    # ---------------- pass 2 ----------------
    with tc.tile_pool(name="p2", bufs=3) as wk3, \
         tc.tile_pool(name="ps_att", bufs=2, space="PSUM") as ps_att, \
         tc.tile_pool(name="ps_cv", bufs=1, space="PSUM") as ps_cv, \
         tc.tile_pool(name="ps_tp", bufs=1, space="PSUM") as ps_tp, \
         tc.tile_pool(name="ps_out", bufs=1, space="PSUM") as ps_out:
        for c in range(CH):
            rows = NSH - (CH - 1) * 128 if c == CH - 1 else 128
            xt = wk3.tile([128, 256], F32, name="xt")
            for h in range(H):
                qsl = qpT_h[h][:, c * 128:(c + 1) * 128]
                pa = ps_att.tile([128, 510], F32, name="pa")
                nc.tensor.matmul(pa[:], lhsT=qsl,
                                 rhs=kvs_rhs_h[h][:, 0:510],
                                 start=True, stop=True)
                pb = ps_att.tile([128, 140], F32, name="pb")
                nc.tensor.matmul(pb[:], lhsT=qsl,
                                 rhs=kvs_rhs_h[h][:, 510:650],
                                 start=True, stop=True)
                rec = wk3.tile([128, 10], F32, name="rec")
                nc.vector.reciprocal(rec[:], pb[:, 130:140])
                nc.vector.tensor_scalar(rec[:], rec[:], 1.0 / K, None,
                                        op0=ALU.mult)
                zoa = wk3.tile([128, 510], F32, name="zoa")
                nc.vector.tensor_tensor(
                    zoa[:].rearrange("p (d k) -> p d k", k=10),
                    pa[:].rearrange("p (d k) -> p d k", k=10),
                    rec[:].rearrange("p (o k) -> p o k", o=1)
                          .to_broadcast([128, 51, 10]),
                    op=ALU.mult)
                zob = wk3.tile([128, 130], F32, name="zob")
                nc.vector.tensor_tensor(
                    zob[:].rearrange("p (d k) -> p d k", k=10),
                    pb[:, 0:130].rearrange("p (d k) -> p d k", k=10),
                    rec[:].rearrange("p (o k) -> p o k", o=1)
                          .to_broadcast([128, 13, 10]),
                    op=ALU.mult)
                nc.vector.tensor_reduce(
                    xt[:, h * 64:h * 64 + 51],
                    zoa[:].rearrange("p (d k) -> p d k", k=10),
                    axis=AX.X, op=ALU.add)
                nc.vector.tensor_reduce(
                    xt[:, h * 64 + 51:(h + 1) * 64],
                    zob[:].rearrange("p (d k) -> p d k", k=10),
                    axis=AX.X, op=ALU.add)
            # ---- edge conv for window c
            pc = ps_cv.tile([128, 256], F32, name="pc")
            # one-hot built from degree prefix ranges: st[p,col] = 1 iff
            # cumlo[col] <= slot < cumhi[col], slot = cc*128 + p, via
            # clamp01(slot - cum + 1) steps (known-good max/min ALU ops)
            eclo = wk3.tile([1, 128], I16, name="eclo")
            nc.sync.dma_start(eclo[:], io["ecum"][c:c + 1, :])
            echi = wk3.tile([1, 128], I16, name="echi")
            nc.sync.dma_start(echi[:], io["ecum"][CH + c:CH + c + 1, :])
            eclof = wk3.tile([1, 128], F32, name="eclof")
            nc.vector.tensor_copy(eclof[:], eclo[:])
            echif = wk3.tile([1, 128], F32, name="echif")
            nc.vector.tensor_copy(echif[:], echi[:])
            culo_b = wk3.tile([128, 128], F32, name="culo_b")
            nc.gpsimd.partition_broadcast(culo_b[:], eclof[:], channels=128)
            cuhi_b = wk3.tile([128, 128], F32, name="cuhi_b")
            nc.gpsimd.partition_broadcast(cuhi_b[:], echif[:], channels=128)
            culo2 = wk3.tile([128, 128], F32, name="culo2")
            nc.vector.tensor_tensor(
                culo2[:], iota_pf[:, 0:1].to_broadcast([128, 128]),
                culo_b[:], op=ALU.subtract)
            cuhi2 = wk3.tile([128, 128], F32, name="cuhi2")
            nc.vector.tensor_tensor(
                cuhi2[:], iota_pf[:, 0:1].to_broadcast([128, 128]),
                cuhi_b[:], op=ALU.subtract)
            er16 = wk3.tile([128, cw[c]], U16, name="er16")
            nc.sync.dma_start(er16[:], io["erow"][:, off[c]:off[c + 1]])
            ert = wk3.tile([128, cw[c]], I32, name="ert")
            nc.vector.tensor_copy(ert[:], er16[:])
            for cc in range(cw[c]):
                sta = wk3.tile([128, 128], F32, name="sta")
                nc.vector.tensor_scalar(sta[:], culo2[:], float(cc * 128),
                                        0.0, op0=ALU.add, op1=ALU.max)
                nc.vector.tensor_scalar(sta[:], sta[:], 1.0, None,
                                        op0=ALU.min)
                stb = wk3.tile([128, 128], F32, name="stb")
                nc.vector.tensor_scalar(stb[:], cuhi2[:], float(cc * 128),
                                        0.0, op0=ALU.add, op1=ALU.max)
                nc.vector.tensor_scalar(stb[:], stb[:], 1.0, None,
                                        op0=ALU.min)
                st = wk3.tile([128, 128], F32, name="st")
                nc.vector.tensor_tensor(st[:], sta[:], stb[:],
                                        op=ALU.subtract)
                vg = wk3.tile([128, 258], U8, name="vg")
                nc.gpsimd.indirect_dma_start(
                    out=vg[:], out_offset=None, in_=vtab_full[:],
                    in_offset=bass.IndirectOffsetOnAxis(ap=ert[:, cc:cc + 1],
                                                        axis=0))
                vgf = wk3.tile([128, 256], F32, name="vgf")
                nc.vector.tensor_copy(vgf[:], vg[:, 0:256].bitcast(I8))
                vgs = wk3.tile([128, 1], F32, name="vgs")
                nc.vector.tensor_copy(vgs[:], vg[:, 256:258].bitcast(BF16))
                nc.vector.tensor_scalar(vgf[:], vgf[:], vgs[:, 0:1],
                                        1.0 / 127.0, op0=ALU.mult,
                                        op1=ALU.mult)
                nc.tensor.matmul(pc[:], lhsT=st[:], rhs=vgf[:],
                                 start=(cc == 0), stop=(cc == cw[c] - 1))
            rsib = wk3.tile([128, 1], BF16, name="rsib")
            if rows < 128:
                nc.gpsimd.memset(rsib[:], 0.0)
            nc.sync.dma_start(rsib[0:rows, :],
                              io["rsio"][c * 128:c * 128 + rows, 0:1])
            rsi = wk3.tile([128, 1], F32, name="rsi")
            nc.vector.tensor_copy(rsi[:], rsib[:])
            x2 = wk3.tile([128, 256], F32, name="x2")
            nc.vector.tensor_scalar(x2[:], pc[:], rsi[:, 0:1], None,
                                    op0=ALU.mult)
            nc.vector.tensor_tensor(x2[:], x2[:], sigb, op=ALU.mult)
            nc.vector.tensor_add(xt[:], xt[:], x2[:])
            # ---- output projection
            tp0 = ps_tp.tile([128, 128], F32, name="tp0")
            nc.tensor.transpose(tp0[:], xt[:, 0:128], ident[:])
            tp1 = ps_tp.tile([128, 128], F32, name="tp1")
            nc.tensor.transpose(tp1[:], xt[:, 128:256], ident[:])
            xt0 = wk3.tile([128, 128], F32, name="xt0")
            nc.vector.tensor_copy(xt0[:], tp0[:])
            xt1 = wk3.tile([128, 128], F32, name="xt1")
            nc.vector.tensor_copy(xt1[:], tp1[:])
            po = ps_out.tile([128, 64], F32, name="po")
            nc.tensor.matmul(po[:], lhsT=xt0[:], rhs=woT0[:],
                             start=True, stop=False)
            nc.tensor.matmul(po[:], lhsT=xt1[:], rhs=woT1[:],
                             start=False, stop=True)
            osb = wk3.tile([128, 64], F32, name="osb")
            nc.vector.tensor_tensor(osb[:], po[:], woscl, op=ALU.mult)
            nc.vector.tensor_add(osb[:], osb[:], wob)
            # int8 output quantization with per-row (node) scale
            aabs = wk3.tile([128, 64], F32, name="aabs")
            nc.scalar.activation(aabs[:], osb[:], ACT.Abs)
            am = wk3.tile([128, 1], F32, name="am")
            nc.vector.tensor_reduce(am[:], aabs[:], axis=AX.X, op=ALU.max)
            nc.vector.tensor_scalar(am[:], am[:], 1e-30, None, op0=ALU.add)
            # round the scale to bf16 FIRST so quant and host dequant agree
            amb = wk3.tile([128, 1], BF16, name="amb")
            nc.vector.tensor_copy(amb[:], am[:])
            am2 = wk3.tile([128, 1], F32, name="am2")
            nc.vector.tensor_copy(am2[:], amb[:])
            rec = wk3.tile([128, 1], F32, name="orec")
            nc.vector.reciprocal(rec[:], am2[:])
            nc.vector.tensor_scalar(rec[:], rec[:], 127.0, None, op0=ALU.mult)
            oq = wk3.tile([128, 64], F32, name="oq")
            nc.vector.tensor_scalar(oq[:], osb[:], rec[:, 0:1], None,
                                    op0=ALU.mult)
            ob = wk3.tile([128, 64], I8, name="ob")
            nc.vector.tensor_copy(ob[:], oq[:])
            nc.sync.dma_start(
                out_d[c * 128:c * 128 + rows, 0:64].bitcast(I8), ob[0:rows, :])
            nc.sync.dma_start(
                out_d[c * 128:c * 128 + rows, 64:66].bitcast(BF16),
                amb[0:rows, :])


# ------------------------------------------------------------- cached runner
class _Runner:
    """One compiled jit(shard_map(bass_exec)) executable, reused per call."""

    def __init__(self, cw, off, cwt):
        import jax
        from jax.sharding import Mesh, PartitionSpec, NamedSharding
        from jax.experimental.shard_map import shard_map
        from concourse.bass2jax import (_bass_exec_p, install_neuronx_cc_hook,
                                        partition_id_tensor)

        install_neuronx_cc_hook()
        try:
            # strip source paths from HLO metadata so the PJRT NEFF cache
            # key is independent of where kernel.py lives
            jax.config.update("jax_hlo_source_file_canonicalization_regex",
                              ".*")
        except Exception:
            pass
        nc = bacc.Bacc("TRN2", target_bir_lowering=False, debug=False,
                       enable_asserts=False, num_devices=NCORE)
        with tile.TileContext(nc) as tc:
            with ExitStack() as ctx:
                _build(nc, tc, ctx, cw, off, cwt)
        nc.compile()
        self.nc = nc

        partition_name = nc.partition_id_tensor.name if nc.partition_id_tensor else None
        in_names, out_names, out_avals = [], [], []
        for alloc in nc.m.functions[0].allocations:
            if not isinstance(alloc, mybir.MemoryLocationSet):
                continue
            name = alloc.memorylocations[0].name
            if alloc.kind == "ExternalInput":
                if name != partition_name:
                    in_names.append(name)
            elif alloc.kind == "ExternalOutput":
                out_avals.append(jax.core.ShapedArray(
                    tuple(alloc.tensor_shape), mybir.dt.np(alloc.dtype)))
                out_names.append(name)
        in_names_full = in_names + out_names + (
            [partition_name] if partition_name else [])
        self.in_names = in_names
        self.out_names = out_names

        def _body(*args):
            operands = list(args)
            if partition_name is not None:
                operands.append(partition_id_tensor())
            return tuple(_bass_exec_p.bind(
                *operands, out_avals=tuple(out_avals),
                in_names=tuple(in_names_full), out_names=tuple(out_names),
                lowering_input_output_aliases=(),
                sim_require_finite=True, sim_require_nnan=True, nc=nc))

        devices = jax.devices()[:NCORE]
        mesh = Mesh(np.asarray(devices), ("core",))
        nargs = len(in_names) + len(out_names)
        self._jitted = jax.jit(
            shard_map(_body, mesh=mesh,
                      in_specs=(PartitionSpec("core"),) * nargs,
                      out_specs=(PartitionSpec("core"),) * len(out_names),
                      check_rep=False),
            keep_unused=True)
        sh = NamedSharding(mesh, PartitionSpec("core"))
        self._zero_args = [
            jax.device_put(
                np.zeros((NCORE * a.shape[0], *a.shape[1:]), a.dtype), sh)
            for a in out_avals]
        for z in self._zero_args:
            z.block_until_ready()
        from concurrent.futures import ThreadPoolExecutor
        self._pool = ThreadPoolExecutor(18)

    def _fetch(self, out) -> np.ndarray:
        """Parallel per-shard device->host fetch, reassembled in core order."""
        shards = sorted(out.addressable_shards,
                        key=lambda s: (s.index[0].start or 0))
        parts = list(self._pool.map(lambda s: np.asarray(s.data), shards))
        return np.concatenate(parts, axis=0)

    def run(self, arrs) -> np.ndarray:
        """One retry: the axon tunnel can throw transient INTERNAL errors on
        fetch under sustained load; a clean second attempt recovers."""
        try:
            return self._run_once(arrs)
        except Exception:
            return self._run_once(arrs)

    def _run_once(self, arrs) -> np.ndarray:
        """arrs: name -> global concatenated array. Returns full [B,N,64] f32.

        Each fetch thread dequantizes its own shard as soon as its bytes
        arrive, overlapping host work with the other shards' transfers."""
        args = [arrs[nm] for nm in self.in_names]
        outs = self._jitted(*args, *self._zero_args)
        shards = sorted(outs[0].addressable_shards,
                        key=lambda s: (s.index[0].start or 0))
        res = np.empty((N, 64), np.float32)

        def work(item):
            i, shd = item
            buf = np.asarray(shd.data)                     # [NSH, 66] u8
            q = buf[:, :64].view(np.int8)
            s = np.ascontiguousarray(buf[:, 64:66]).view(bfloat16)
            s = s.astype(np.float32)
            np.multiply(q.astype(np.float32), s * (1.0 / 127.0),
                        out=res[i * NSH:(i + 1) * NSH])

        list(self._pool.map(work, enumerate(shards)))
        return res.reshape(B, N, 64)


_CACHE = {}


def _get_runner(cw, off, cwt):
    key = (cwt, tuple(cw))
    if key not in _CACHE:
        _CACHE[key] = _Runner(cw, off, cwt)
    return _CACHE[key]


def kernel(**inputs) -> np.ndarray:
    inputs = {k: np.asarray(v) for k, v in inputs.items()}
    arrs, cw, off, cwt = _prep(**inputs)
    runner = _get_runner(cw, off, cwt)
    return runner.run(arrs)


# revision 92
# speedup vs baseline: 1.1041x; 1.1041x over previous
"""NodeFormerConv on 8 TRN2 cores (axon-tunneled).

The wall-clock of a call is dominated by the axon wire (host->device input
transfer at ~70-90 MB/s, output fetch at ~40-50 MB/s, ~80 ms RPC floor) and
by per-call jax.jit retrace/compile when going through
bass_utils.run_bass_kernel_spmd.  So the layout here is:

 * one cached jit(shard_map(bass_exec)) executable per edge-layout key --
   no retrace, no XLA/neuronx recompile, zero output buffers kept
   device-resident (not donated, so they are reusable),
 * wire-compressed inputs: z int8 (per-node scale), exp(gumbels) uint8
   (per-node-head scale), weights int8 (per-out-channel scale), one-hot
   edge columns uint8, edge row ids uint16, bias/sigmoid rows
   de-duplicated to [1,*] rows,
 * packed int8 output (64B payload + 4B f32 per-node scale per row),
   fetched shard-parallel and dequantized on host,
 * the device kernel is the same math as the f32 baseline with a small
   dequantize prologue (device compute is ~free next to the wire).
   End-to-end rel err ~1e-2 against the f32 reference (gate is 2e-2).

Sharding: node dim N=30000 -> 3750/core (padded 3840 = 30 chunks of 128).
Pass 1a: q/k/v projections, qp (local stab), dd_k stored (diag folded),
         local key-stab partials, v-table write.
Collectives: AllReduce-max key stab [1,4]; AllGather v-table [30000,256].
Pass 1b: kp=exp, KG=kp*g, kvs/ks_sum accumulation (PE, ones-column trick).
Collective: AllReduce-add kvs [260,300]; reshuffle to [30m, (d,k)+ks].
Pass 2:  z_num/z_den matmuls, divide+mean over K, edge conv via one-hot
         scatter matmul over indirect-gathered v rows, output projection.
"""

import math
from contextlib import ExitStack

import numpy as np
from ml_dtypes import bfloat16

import concourse.bass as bass
import concourse.tile as tile
from concourse import mybir, bacc, bass_isa
from concourse.masks import make_identity

F32 = mybir.dt.float32
BF16 = mybir.dt.bfloat16
I32 = mybir.dt.int32
U16 = mybir.dt.uint16
U8 = mybir.dt.uint8
I8 = mybir.dt.int8
I16 = mybir.dt.int16
AX = mybir.AxisListType
ALU = mybir.AluOpType
ACT = mybir.ActivationFunctionType

B, N, CIN, H, D, M, K, E = 1, 30000, 128, 4, 64, 30, 10, 480000
NCORE = 8
NSH = N // NCORE            # 3750
CH = 30                     # chunks per core
NPAD = CH * 128             # 3840
TAU = 0.25
EPS = 1e-6
ALPHA = (float(D) ** -0.25) * (TAU ** -0.5)   # folded into P
RATIO = float(M) ** -0.5
PADCOL = 200                # one-hot miss sentinel for pad edges (u8)
# device-gathered weight stream: wqkvT | woT | cst | brows (64B-aligned)
WOFF_WQKV, WOFF_WO, WOFF_CST, WOFF_BROWS = 0, 98304, 114688, 151552
WTOT = 155136
WSLICE = WTOT // NCORE      # 19392


# --------------------------------------------------------------- blob layout
def _layout(cwt):
    """Single packed u8 input blob per core: one wire transfer instead of 11
    (measured ~3ms per-arg staging overhead on the axon tunnel)."""
    entries = [
        # weights ride as one distinct 1/8 slice per core and are AllGathered
        # on device (they were the last 8x-replicated wire bytes)
        ("wslice", (1, WSLICE),    np.uint8,   U8),
        ("zscl",  (1, NSH),        bfloat16,   BF16),
        ("gscl",  (NSH, H),        np.int8,    I8),
        ("rsio",  (NSH, 2),        bfloat16,   BF16),
        ("erow",  (128, cwt),      np.uint16,  U16),
        ("zT",    (128, NSH),      np.int8,    I8),
        ("gexp",  (NSH, H * K),    np.uint8,   U8),
        # per-window exclusive/inclusive degree prefix tables (minus 1, for
        # the clamp01 step trick); replaces the expanded one-hot ecol table
        ("ecum",  (2 * CH, 128),   np.int16,   I16),
    ]
    lay, off = {}, 0
    for nm, shp, npdt, birdt in entries:
        nb = int(np.prod(shp)) * np.dtype(npdt).itemsize
        off = (off + 63) // 64 * 64
        lay[nm] = (off, nb, shp, npdt, birdt)
        off += nb
    totb = (off + 127) // 128 * 128
    return lay, totb


# ----------------------------------------------------------------- host prep
def _prep(z, edge_index, Wq_w, Wq_b, Wk_w, Wk_b, Wv_w, Wv_b, Wo_w, Wo_b, b,
          projection_matrix, gumbels):
    """Returns (arrs, cw, off, cwt): arrs = {"blob": [NCORE, totb] u8}.

    The edge-layout key (cwt) is derived first so the blob can be allocated
    up front; the z / gumbel / edge sections are then quantized by worker
    threads writing directly into views of the blob."""
    from concurrent.futures import ThreadPoolExecutor

    col32 = np.asarray(edge_index[1], np.int32)
    row32 = np.asarray(edge_index[0], np.int32)
    c_of0 = col32 // NSH
    local0 = col32 - c_of0 * NSH
    w_of0 = local0 // 128
    blk0 = (c_of0 * CH + w_of0).astype(np.uint8)
    ec = np.bincount(blk0, minlength=NCORE * CH).reshape(NCORE, CH)
    cw = [max(1, int(math.ceil(ec[:, w].max() / 128.0))) for w in range(CH)]
    off = np.cumsum([0] + cw)
    cwt = int(off[-1])

    lay, totb = _layout(cwt)
    blob = np.zeros((NCORE, totb), np.uint8)

    def sect(nm):
        boff, nb, shp, npdt, _ = lay[nm]
        return blob[:, boff:boff + nb].view(npdt).reshape((NCORE,) + shp)

    def work_z():
        z2 = np.asarray(z, np.float32).reshape(N, CIN)
        # int8 per-node quantization: round(z_n / s_n), s_n = max|z_n|/127
        s = np.maximum(np.abs(z2).max(axis=1), 1e-30) / 127.0    # [N]
        t = z2 * (1.0 / s)[:, None]                              # [N,128]
        np.rint(t, out=t)   # exact ints in [-127,127]; u8-view assign casts
        sect("zT")[:] = t.reshape(NCORE, NSH, CIN).transpose(0, 2, 1)
        sect("zscl")[:, 0, :] = s.reshape(NCORE, NSH).astype(bfloat16)

    def work_g():
        # exp(gumbels) quantized u8 with per-(node,head) power-of-2 scale
        # 2^e, e = ceil(log2(max/255)) shipped as int8; rint(g/2^e) <= 255
        # exactly since 2^e >= max/255
        g2 = np.exp(np.asarray(gumbels, np.float32)).reshape(N, H, K)
        gs = np.maximum(g2.max(axis=2), 1e-30)                   # [N,H]
        e = np.ceil(np.log2(gs / 255.0)).astype(np.float32)
        np.divide(g2, np.exp2(e)[..., None], out=g2)
        np.rint(g2, out=g2)
        sect("gexp")[:] = g2.reshape(NCORE, NSH, H * K)
        sect("gscl")[:] = e.reshape(NCORE, NSH, H)   # exact ints -> i8 cast

    def work_e():
        d_in = np.bincount(col32, minlength=N).astype(np.float32)
        d_out = np.bincount(row32, minlength=N).astype(np.float32)
        rsio = sect("rsio")
        rsio[:, :, 0] = (1.0 / np.sqrt(np.maximum(d_in, 1.0))
                         ).reshape(NCORE, NSH).astype(bfloat16)
        rsio[:, :, 1] = (1.0 / np.sqrt(np.maximum(d_out, 1.0))
                         ).reshape(NCORE, NSH).astype(bfloat16)
        # sort edges by column (u16 radix): groups by (core, window) AND
        # orders by column within each window, so slots form contiguous
        # per-column runs describable by degree prefix ranges
        order = np.argsort(col32.astype(np.uint16), kind="stable")
        rs, cs = row32[order], col32[order]
        c_of = cs // NSH
        local = cs - c_of * NSH
        w_of = local // 128
        blk = c_of * CH + w_of                  # sorted ascending
        starts = np.zeros(NCORE * CH, np.int64)
        np.cumsum(ec.reshape(-1)[:-1], out=starts[1:])
        slot = np.arange(E, dtype=np.int64) - starts[blk]
        off_arr = np.asarray(off[:-1], np.int64)
        pcol = off_arr[w_of] + slot // 128
        prow = slot % 128
        erow = sect("erow")
        # +row//NSH: v-table rows are gathered as 8 blocks of NSH+1 (row NSH
        # of each block carries stab partials), so global node id r lives at
        # gathered row r + r//NSH
        erow[c_of, prow, pcol] = (rs + rs // NSH).astype(np.uint16)
        # per-window degree prefix tables: cumlo-1 rows 0..CH-1, cumhi-1
        # rows CH..2CH-1; window w covers nodes [w*128, w*128+128) of the
        # core shard (zero-degree pad cols in the last window)
        degs = np.zeros((NCORE, CH * 128), np.int64)
        degs[:, :NSH] = d_in.astype(np.int64).reshape(NCORE, NSH)
        degs = degs.reshape(NCORE, CH, 128)
        cumhi = np.cumsum(degs, axis=2)
        cumlo = cumhi - degs
        ecum = sect("ecum").reshape(NCORE, 2, CH, 128)
        ecum[:, 0] = cumlo - 1
        ecum[:, 1] = cumhi - 1

    pool = ThreadPoolExecutor(3)
    futs = [pool.submit(w) for w in (work_z, work_g, work_e)]

    # weights quantized i8 with per-output-channel scale
    def _qw(w):
        w = np.asarray(w, np.float32)
        s = np.maximum(np.abs(w).max(axis=1), 1e-30) / 127.0    # [cout]
        q = np.rint(w / s[:, None]).astype(np.int8)
        return q.T, s.astype(np.float32)                        # [cin,cout], [cout]

    wqT, sq = _qw(Wq_w)
    wkT, sk = _qw(Wk_w)
    wvT, sv = _qw(Wv_w)
    woTq, so = _qw(Wo_w)
    wstream = np.zeros(WTOT, np.uint8)
    wstream[WOFF_WQKV:WOFF_WO].view(np.int8).reshape(3, 128, 256)[:] = \
        np.stack([wqT, wkT, wvT])
    wstream[WOFF_WO:WOFF_CST].view(np.int8).reshape(2, 128, 64)[:] = \
        np.stack([woTq[:128], woTq[128:]])

    # cst [128,72] f32: qkb(4) | pT2(60) | nh2(2) | q/k half scales(4) | pad(2)
    cst = np.zeros((128, 72), np.float32)
    cst[:, 0] = Wq_b[:128]
    cst[:, 1] = Wq_b[128:]
    cst[:, 2] = Wk_b[:128]
    cst[:, 3] = Wk_b[128:]
    pT = (ALPHA * np.asarray(projection_matrix, np.float32)).T  # [64,30]
    cst[0:64, 4:4 + M] = pT
    cst[64:128, 4 + M:4 + 2 * M] = pT
    cst[0:64, 64] = -0.5
    cst[64:128, 65] = -0.5
    cst[:, 66] = sq[:128]
    cst[:, 67] = sq[128:]
    cst[:, 68] = sk[:128]
    cst[:, 69] = sk[128:]
    wstream[WOFF_CST:WOFF_BROWS].view(np.float32).reshape(128, 72)[:] = cst

    # brows [1,896] f32: vb(256) | wob(64) | sig(256) | v scales(256) | wo scales(64)
    sig = 1.0 / (1.0 + np.exp(-np.asarray(b, np.float64)[0]))   # [H]
    brows = np.zeros((1, 896), np.float32)
    brows[0, 0:256] = np.asarray(Wv_b, np.float32)
    brows[0, 256:320] = np.asarray(Wo_b, np.float32)
    brows[0, 320:576] = np.repeat(sig.astype(np.float32), 64)
    brows[0, 576:832] = sv
    brows[0, 832:896] = so
    wstream[WOFF_BROWS:WTOT].view(np.float32).reshape(1, 896)[:] = brows
    sect("wslice")[:] = wstream.reshape(NCORE, 1, WSLICE)

    for f in futs:
        f.result()
    pool.shutdown(wait=False)
    return {"blob": blob}, cw, [int(x) for x in off], cwt


# ------------------------------------------------------------- device build
def _build(nc, tc, ctx, cw, off, cwt):
    lay, totb = _layout(cwt)
    blob = nc.dram_tensor("blob", [1, totb], U8, kind="ExternalInput").ap()
    io = {}
    for nm, (boff, nb, shp, npdt, birdt) in lay.items():
        seg = blob[0:1, boff:boff + nb].bitcast(birdt)
        if shp[0] == 1:
            io[nm] = seg
        else:
            io[nm] = seg.rearrange("o (r c) -> (o r) c", c=shp[1])
    # packed output: per node row = 64 bytes int8 payload + 2 bytes bf16 scale
    out_d = nc.dram_tensor("out", [NSH, 66], U8, kind="ExternalOutput").ap()

    dram = ctx.enter_context(tc.tile_pool(name="dram", bufs=1, space="DRAM"))
    # v-table in bf16: halves the [N,256] AllGather volume (the dominant
    # collective); costs ~0.2% on the conv term, well inside the 2e-2 gate.
    # Row NSH of each core's table carries its key-stab partials, fusing the
    # stab AllReduce-max into this AllGather (a common per-head stab constant
    # cancels in z_num/z_den, so its exact value only gates exp overflow).
    vtab_loc = dram.tile([NSH + 1, H * D], BF16)
    vtab_full = dram.tile([NCORE * (NSH + 1), H * D], BF16, addr_space="Shared")
    kvs_in = dram.tile([H * 65, 300], F32)
    kvs_out = dram.tile([H * 65, 300], F32, addr_space="Shared")
    wsl_in = dram.tile([1, WSLICE // 4], F32)
    wfull = dram.tile([1, WTOT // 4], F32, addr_space="Shared")

    const = ctx.enter_context(tc.tile_pool(name="const", bufs=1))
    big = ctx.enter_context(tc.tile_pool(name="big", bufs=1))

    # ---- decompress prologue: bf16/u8/u16 -> f32 working tiles
    with tc.tile_pool(name="stage", bufs=1) as stage:
        # gather the 8 per-core weight slices into the full weight stream
        wsl_sb = stage.tile([1, WSLICE], U8, name="wsl_sb")
        nc.sync.dma_start(wsl_sb[:], io["wslice"])
        nc.sync.dma_start(wsl_in[:], wsl_sb[:].bitcast(F32))
        nc.gpsimd.collective_compute(
            "AllGather", ALU.bypass, replica_groups=[list(range(NCORE))],
            ins=[wsl_in[:].opt()], outs=[wfull[:].opt()])
        wb = wfull[:].bitcast(U8)                      # [1, WTOT]
        io["wqkvT"] = wb[0:1, WOFF_WQKV:WOFF_WO].bitcast(I8).rearrange(
            "o (s r c) -> (o s) r c", s=3, r=128)
        io["woT"] = wb[0:1, WOFF_WO:WOFF_CST].bitcast(I8).rearrange(
            "o (s r c) -> (o s) r c", s=2, r=128)
        io["cst"] = wb[0:1, WOFF_CST:WOFF_BROWS].bitcast(F32).rearrange(
            "o (r c) -> (o r) c", c=72)
        io["brows"] = wb[0:1, WOFF_BROWS:WTOT].bitcast(F32)
        wq = const.tile([128, 256], F32)
        wk = const.tile([128, 256], F32)
        wv = const.tile([128, 256], F32)
        for wdst, idx in ((wq, 0), (wk, 1), (wv, 2)):
            wbf = stage.tile([128, 256], I8, name=f"wbf{idx}")
            nc.sync.dma_start(wbf[:], io["wqkvT"][idx])
            nc.vector.tensor_copy(wdst[:], wbf[:])
        woT0 = const.tile([128, 64], F32)
        woT1 = const.tile([128, 64], F32)
        for wdst, idx in ((woT0, 0), (woT1, 1)):
            wbf = stage.tile([128, 64], I8, name=f"obf{idx}")
            nc.sync.dma_start(wbf[:], io["woT"][idx])
            nc.vector.tensor_copy(wdst[:], wbf[:])
        cst = const.tile([128, 72], F32)
        nc.sync.dma_start(cst[:], io["cst"][:])
        brow_sb = stage.tile([1, 896], F32, name="brow_sb")
        nc.sync.dma_start(brow_sb[:], io["brows"][:])
        bb = const.tile([128, 896], F32)
        nc.gpsimd.partition_broadcast(bb[:], brow_sb[:], channels=128)
        zT = big.tile([128, NPAD], F32)
        zbf = stage.tile([128, NPAD], I8, name="zbf")
        nc.gpsimd.memset(zbf[:, NSH:NPAD], 0.0)
        nc.sync.dma_start(zbf[:, 0:NSH], io["zT"][:])
        nc.vector.tensor_copy(zT[:], zbf[:])
        zs_rb = stage.tile([1, NPAD], BF16, name="zs_rb")
        nc.gpsimd.memset(zs_rb[:, NSH:NPAD], 0.0)
        nc.sync.dma_start(zs_rb[:, 0:NSH], io["zscl"][:])
        zs_row = stage.tile([1, NPAD], F32, name="zs_row")
        nc.vector.tensor_copy(zs_row[:], zs_rb[:])
        zs_b = stage.tile([128, NPAD], F32, name="zs_b")
        nc.gpsimd.partition_broadcast(zs_b[:], zs_row[:], channels=128)
        nc.vector.tensor_tensor(zT[:], zT[:], zs_b[:], op=ALU.mult)
    qkb = cst[:, 0:4]
    pT2 = cst[:, 4:64]
    nh2 = cst[:, 64:66]
    wsqk = cst[:, 66:70]          # per-partition q/k out-channel scales (halves)
    vb = bb[:, 0:256]
    wob = bb[:, 256:320]
    sigb = bb[:, 320:576]
    vscl = bb[:, 576:832]
    woscl = bb[:, 832:896]

    ident = const.tile([128, 128], F32)
    make_identity(nc, ident[:])
    iota_i = const.tile([128, 128], I32)
    nc.gpsimd.iota(iota_i[:], pattern=[[1, 128]], base=0, channel_multiplier=0)
    iota_f = const.tile([128, 128], F32)
    nc.vector.tensor_copy(iota_f[:], iota_i[:])
    with tc.tile_pool(name="ps_it", bufs=1, space="PSUM") as ps_it:
        itp = ps_it.tile([128, 128], F32)
        nc.tensor.transpose(itp[:], iota_f[:], ident[:])
        iota_pf = const.tile([128, 1], F32)     # [p,0] = p
        nc.vector.tensor_copy(iota_pf[:], itp[:, 0:1])

    qpT_h = [big.tile([30, NPAD], F32, name=f"qpT{h}") for h in range(H)]
    dd_all = big.tile([128, H * M * CH], F32)       # col = h*900 + c*30
    v_all = big.tile([128, CH * 260], F32)          # per chunk [65*4]
    stabpart = big.tile([128, 4 * CH], F32)         # col = c*4 + (2*half+hh)
    nc.gpsimd.memset(stabpart[:], -1e30)
    kvs_rhs_h = [big.tile([30, 650], F32, name=f"kvsr{h}") for h in range(H)]

    # ---------------- pass 1a ----------------
    with tc.tile_pool(name="p1a", bufs=3) as wk1, \
         tc.tile_pool(name="ps_qkv", bufs=2, space="PSUM") as ps_qkv, \
         tc.tile_pool(name="ps_sm", bufs=1, space="PSUM") as ps_sm:
        for c in range(CH):
            rows = NSH - c * 128 if c == CH - 1 else 128
            zsl = zT[:, c * 128:(c + 1) * 128]
            for qi, (wmat, bcol0) in enumerate([(wq, 0), (wk, 2)]):
                for hf in range(2):
                    qps = ps_qkv.tile([128, 128], F32, name="qps")
                    nc.tensor.matmul(qps[:], lhsT=wmat[:, hf * 128:(hf + 1) * 128],
                                     rhs=zsl, start=True, stop=True)
                    qsb = wk1.tile([128, 128], F32, name="qsb")
                    nc.vector.tensor_scalar(
                        qsb[:], qps[:], wsqk[:, bcol0 + hf:bcol0 + hf + 1],
                        qkb[:, bcol0 + hf:bcol0 + hf + 1],
                        op0=ALU.mult, op1=ALU.add)
                    sq = wk1.tile([128, 128], F32, name="sq")
                    nc.scalar.activation(sq[:], qsb[:], ACT.Square, scale=ALPHA)
                    dg = ps_sm.tile([128, 2], F32, name="dg")
                    nc.tensor.matmul(dg[:], lhsT=sq[:], rhs=nh2[:],
                                     start=True, stop=True)
                    dd = ps_sm.tile([128, 60], F32, name="dd")
                    nc.tensor.matmul(dd[:], lhsT=qsb[:], rhs=pT2[:],
                                     start=True, stop=True)
                    smax = wk1.tile([128, 2], F32, name="smax")
                    nc.vector.tensor_reduce(
                        smax[:], dd[:].rearrange("p (h m) -> p h m", h=2),
                        axis=AX.X, op=ALU.max)
                    if qi == 0:  # ---- query: exp with local stab
                        bias2 = wk1.tile([128, 2], F32, name="bias2")
                        nc.vector.tensor_tensor(bias2[:], dg[:], smax[:],
                                                op=ALU.subtract)
                        qp2 = wk1.tile([128, 60], F32, name="qp2")
                        for hh in range(2):
                            nc.scalar.activation(
                                qp2[:, hh * 30:(hh + 1) * 30],
                                dd[:, hh * 30:(hh + 1) * 30], ACT.Exp,
                                bias=bias2[:, hh:hh + 1])
                        nc.vector.tensor_scalar(qp2[:], qp2[:], EPS, RATIO,
                                                op0=ALU.add, op1=ALU.mult)
                        for hh in range(2):
                            tpq = ps_sm.tile([30, 128], F32, name="tpq")
                            nc.tensor.transpose(
                                tpq[:], qp2[:, hh * 30:(hh + 1) * 30],
                                ident[:])
                            nc.vector.tensor_copy(
                                qpT_h[hf * 2 + hh][:, c * 128:(c + 1) * 128],
                                tpq[:])
                    else:  # ---- key: store stab partials + dd' (diag folded)
                        nc.vector.tensor_copy(
                            stabpart[0:rows, c * 4 + hf * 2:c * 4 + hf * 2 + 2],
                            smax[0:rows, :])
                        dgs = wk1.tile([128, 2], F32, name="dgs")
                        nc.vector.tensor_copy(dgs[:], dg[:])
                        for hh in range(2):
                            h = hf * 2 + hh
                            nc.scalar.activation(
                                dd_all[:, h * (M * CH) + c * M:
                                       h * (M * CH) + (c + 1) * M],
                                dd[:, hh * 30:(hh + 1) * 30], ACT.Identity,
                                bias=dgs[:, hh:hh + 1])
            # ---- v (node-major)
            vps = ps_qkv.tile([128, 256], F32, name="vps")
            nc.tensor.matmul(vps[:], lhsT=zsl, rhs=wv[:], start=True, stop=True)
            vsb = wk1.tile([128, 256], F32, name="vsb")
            nc.vector.tensor_tensor(vsb[:], vps[:], vscl, op=ALU.mult)
            nc.vector.tensor_add(vsb[:], vsb[:], vb)
            nc.gpsimd.memset(v_all[:, c * 260:(c + 1) * 260], 1.0)
            for h in range(H):
                nc.vector.tensor_copy(
                    v_all[:, c * 260 + h * 65:c * 260 + h * 65 + 64],
                    vsb[:, h * 64:(h + 1) * 64])
            rsob = wk1.tile([128, 1], BF16, name="rsob")
            if rows < 128:
                nc.gpsimd.memset(rsob[:], 0.0)
            nc.sync.dma_start(rsob[0:rows, :],
                              io["rsio"][c * 128:c * 128 + rows, 1:2])
            rso = wk1.tile([128, 1], F32, name="rso")
            nc.vector.tensor_copy(rso[:], rsob[:])
            vsc = wk1.tile([128, 256], F32, name="vsc")
            nc.vector.tensor_scalar(vsc[:], vsb[:], rso[:, 0:1], None,
                                    op0=ALU.mult)
            vscb = wk1.tile([128, 256], BF16, name="vscb")
            nc.vector.tensor_copy(vscb[:], vsc[:])
            nc.sync.dma_start(vtab_loc[c * 128:c * 128 + rows, :],
                              vscb[0:rows, :])

    # ------- v-table all-gather (stab partials ride along as row NSH) -----
    with tc.tile_pool(name="stb", bufs=1) as stb:
        stab4 = stb.tile([128, 4], F32)
        nc.vector.tensor_reduce(
            stab4[:], stabpart[:].rearrange("p (c h) -> p h c", h=4),
            axis=AX.X, op=ALU.max)
        stab4r = stb.tile([128, 4], F32)
        nc.gpsimd.partition_all_reduce(stab4r[:], stab4[:], channels=128,
                                       reduce_op=bass_isa.ReduceOp.max)
        stab4b = stb.tile([1, 4], BF16)
        nc.vector.tensor_copy(stab4b[:], stab4r[0:1, :])
        nc.sync.dma_start(vtab_loc[NSH:NSH + 1, 0:4], stab4b[:])
        nc.gpsimd.collective_compute(
            "AllGather", ALU.bypass, replica_groups=[list(range(NCORE))],
            ins=[vtab_loc[:].opt()], outs=[vtab_full[:].opt()])
        # extract the 8 gathered stab rows, reduce max across cores
        srows = vtab_full[:].rearrange("(c r) d -> c r d",
                                       r=NSH + 1)[:, NSH, 0:4]
        stab8b = stb.tile([NCORE, 4], BF16)
        nc.sync.dma_start(stab8b[:], srows)
        stab8 = stb.tile([NCORE, 4], F32)
        nc.vector.tensor_copy(stab8[:], stab8b[:])
        stab8r = stb.tile([NCORE, 4], F32)
        nc.gpsimd.partition_all_reduce(stab8r[:], stab8[:], channels=NCORE,
                                       reduce_op=bass_isa.ReduceOp.max)
        stab_b = big.tile([128, 4], F32)
        nc.gpsimd.partition_broadcast(stab_b[:], stab8r[0:1, :], channels=128)
        negstab = big.tile([128, 4], F32)
        nc.vector.tensor_scalar(negstab[:], stab_b[:], -1.0, None, op0=ALU.mult)

    # ---------------- pass 1b: kvs accumulation ----------------
    with tc.tile_pool(name="p1b", bufs=3) as wk2, \
         tc.tile_pool(name="ps_kvs", bufs=1, space="PSUM") as ps_kvs:
        kvsp = [ps_kvs.tile([65, 300], F32, name=f"kvsp{h}") for h in range(H)]
        for c in range(CH):
            grows = 128 if c < CH - 1 else NSH - (CH - 1) * 128
            gt = wk2.tile([128, 40], U8, name="gt")
            gsb = wk2.tile([128, 4], I8, name="gsb")
            if grows < 128:
                nc.gpsimd.memset(gt[:], 0.0)
                nc.gpsimd.memset(gsb[:], 0.0)
            nc.sync.dma_start(gt[0:grows, :],
                              io["gexp"][c * 128:c * 128 + grows, :])
            nc.sync.dma_start(gsb[0:grows, :],
                              io["gscl"][c * 128:c * 128 + grows, :])
            gsf = wk2.tile([128, 4], F32, name="gsf")
            nc.vector.tensor_copy(gsf[:], gsb[:])
            # decode power-of-2 scale: 2^e = exp(e * ln2)
            nc.scalar.activation(gsf[:], gsf[:], ACT.Exp,
                                 scale=0.6931471805599453)
            ge = wk2.tile([128, 40], F32, name="ge")
            nc.vector.tensor_copy(ge[:], gt[:])
            nc.vector.tensor_tensor(
                ge[:].rearrange("p (h k) -> p h k", h=4),
                ge[:].rearrange("p (h k) -> p h k", h=4),
                gsf[:].rearrange("p (h o) -> p h o", o=1)
                      .to_broadcast([128, 4, 10]),
                op=ALU.mult)
            kp2 = wk2.tile([128, 120], F32, name="kp2")
            for h in range(H):
                nc.scalar.activation(
                    kp2[:, h * 30:(h + 1) * 30],
                    dd_all[:, h * (M * CH) + c * M:h * (M * CH) + (c + 1) * M],
                    ACT.Exp, bias=negstab[:, h:h + 1])
            nc.vector.tensor_scalar(kp2[:], kp2[:], EPS, RATIO,
                                    op0=ALU.add, op1=ALU.mult)
            for h in range(H):
                kg = wk2.tile([128, 300], F32, name="kg")
                nc.vector.tensor_tensor(
                    kg[:].rearrange("p (k m) -> p k m", k=10),
                    kp2[:, h * 30:(h + 1) * 30]
                        .rearrange("p (o m) -> p o m", o=1)
                        .to_broadcast([128, 10, 30]),
                    ge[:, h * 10:(h + 1) * 10]
                        .rearrange("p (k o) -> p k o", o=1)
                        .to_broadcast([128, 10, 30]),
                    op=ALU.mult)
                nc.tensor.matmul(
                    kvsp[h][:], lhsT=v_all[:, c * 260 + h * 65:c * 260 + (h + 1) * 65],
                    rhs=kg[:], start=(c == 0), stop=(c == CH - 1))
        for h in range(H):
            ksb = wk2.tile([65, 300], F32, name="ksb")
            nc.vector.tensor_copy(ksb[:], kvsp[h][:])
            nc.sync.dma_start(kvs_in[h * 65:(h + 1) * 65, :], ksb[:])

    nc.gpsimd.collective_compute(
        "AllReduce", ALU.add, replica_groups=[list(range(NCORE))],
        ins=[kvs_in[:].opt()], outs=[kvs_out[:].opt()])

    # ---------------- kvs reshuffle: [65,(k,m)] -> [30m, (d,k)|ks] --------
    with tc.tile_pool(name="rsh", bufs=2) as rsh, \
         tc.tile_pool(name="ps_rsh", bufs=1, space="PSUM") as ps_rsh:
        for h in range(H):
            kar = rsh.tile([65, 300], F32, name="kar")
            nc.sync.dma_start(kar[:], kvs_out[h * 65:(h + 1) * 65, :])
            for kk in range(K):
                tp = ps_rsh.tile([30, 65], F32, name="tp")
                nc.tensor.transpose(tp[:], kar[:, kk * 30:(kk + 1) * 30],
                                    ident[0:65, 0:65])
                nc.vector.tensor_copy(
                    kvs_rhs_h[h][:, :640]
                        .rearrange("p (d k) -> p d k", k=10)[:, :, kk:kk + 1],
                    tp[:, 0:64].rearrange("p (d o) -> p d o", o=1))
                nc.vector.tensor_copy(
                    kvs_rhs_h[h][:, 640 + kk:641 + kk], tp[:, 64:65])

    # ---------------- pass 2 ----------------
    with tc.tile_pool(name="p2", bufs=3) as wk3, \
         tc.tile_pool(name="ps_att", bufs=2, space="PSUM") as ps_att, \
         tc.tile_pool(name="ps_cv", bufs=1, space="PSUM") as ps_cv, \
         tc.tile_pool(name="ps_tp", bufs=1, space="PSUM") as ps_tp, \
         tc.tile_pool(name="ps_out", bufs=1, space="PSUM") as ps_out:
        for c in range(CH):
            rows = NSH - (CH - 1) * 128 if c == CH - 1 else 128
            xt = wk3.tile([128, 256], F32, name="xt")
            for h in range(H):
                qsl = qpT_h[h][:, c * 128:(c + 1) * 128]
                pa = ps_att.tile([128, 510], F32, name="pa")
                nc.tensor.matmul(pa[:], lhsT=qsl,
                                 rhs=kvs_rhs_h[h][:, 0:510],
                                 start=True, stop=True)
                pb = ps_att.tile([128, 140], F32, name="pb")
                nc.tensor.matmul(pb[:], lhsT=qsl,
                                 rhs=kvs_rhs_h[h][:, 510:650],
                                 start=True, stop=True)
                rec = wk3.tile([128, 10], F32, name="rec")
                nc.vector.reciprocal(rec[:], pb[:, 130:140])
                nc.vector.tensor_scalar(rec[:], rec[:], 1.0 / K, None,
                                        op0=ALU.mult)
                zoa = wk3.tile([128, 510], F32, name="zoa")
                nc.vector.tensor_tensor(
                    zoa[:].rearrange("p (d k) -> p d k", k=10),
                    pa[:].rearrange("p (d k) -> p d k", k=10),
                    rec[:].rearrange("p (o k) -> p o k", o=1)
                          .to_broadcast([128, 51, 10]),
                    op=ALU.mult)
                zob = wk3.tile([128, 130], F32, name="zob")
                nc.vector.tensor_tensor(
                    zob[:].rearrange("p (d k) -> p d k", k=10),
                    pb[:, 0:130].rearrange("p (d k) -> p d k", k=10),
                    rec[:].rearrange("p (o k) -> p o k", o=1)
                          .to_broadcast([128, 13, 10]),
                    op=ALU.mult)
                nc.vector.tensor_reduce(
                    xt[:, h * 64:h * 64 + 51],
                    zoa[:].rearrange("p (d k) -> p d k", k=10),
                    axis=AX.X, op=ALU.add)
                nc.vector.tensor_reduce(
                    xt[:, h * 64 + 51:(h + 1) * 64],
                    zob[:].rearrange("p (d k) -> p d k", k=10),
                    axis=AX.X, op=ALU.add)
            # ---- edge conv for window c
            pc = ps_cv.tile([128, 256], F32, name="pc")
            # one-hot built from degree prefix ranges: st[p,col] = 1 iff
            # cumlo[col] <= slot < cumhi[col], slot = cc*128 + p, via
            # clamp01(slot - cum + 1) steps (known-good max/min ALU ops)
            eclo = wk3.tile([1, 128], I16, name="eclo")
            nc.sync.dma_start(eclo[:], io["ecum"][c:c + 1, :])
            echi = wk3.tile([1, 128], I16, name="echi")
            nc.sync.dma_start(echi[:], io["ecum"][CH + c:CH + c + 1, :])
            eclof = wk3.tile([1, 128], F32, name="eclof")
            nc.vector.tensor_copy(eclof[:], eclo[:])
            echif = wk3.tile([1, 128], F32, name="echif")
            nc.vector.tensor_copy(echif[:], echi[:])
            culo_b = wk3.tile([128, 128], F32, name="culo_b")
            nc.gpsimd.partition_broadcast(culo_b[:], eclof[:], channels=128)
            cuhi_b = wk3.tile([128, 128], F32, name="cuhi_b")
            nc.gpsimd.partition_broadcast(cuhi_b[:], echif[:], channels=128)
            culo2 = wk3.tile([128, 128], F32, name="culo2")
            nc.vector.tensor_tensor(
                culo2[:], iota_pf[:, 0:1].to_broadcast([128, 128]),
                culo_b[:], op=ALU.subtract)
            cuhi2 = wk3.tile([128, 128], F32, name="cuhi2")
            nc.vector.tensor_tensor(
                cuhi2[:], iota_pf[:, 0:1].to_broadcast([128, 128]),
                cuhi_b[:], op=ALU.subtract)
            er16 = wk3.tile([128, cw[c]], U16, name="er16")
            nc.sync.dma_start(er16[:], io["erow"][:, off[c]:off[c + 1]])
            ert = wk3.tile([128, cw[c]], I32, name="ert")
            nc.vector.tensor_copy(ert[:], er16[:])
            for cc in range(cw[c]):
                sta = wk3.tile([128, 128], F32, name="sta")
                nc.vector.tensor_scalar(sta[:], culo2[:], float(cc * 128),
                                        0.0, op0=ALU.add, op1=ALU.max)
                nc.vector.tensor_scalar(sta[:], sta[:], 1.0, None,
                                        op0=ALU.min)
                stb = wk3.tile([128, 128], F32, name="stb")
                nc.vector.tensor_scalar(stb[:], cuhi2[:], float(cc * 128),
                                        0.0, op0=ALU.add, op1=ALU.max)
                nc.vector.tensor_scalar(stb[:], stb[:], 1.0, None,
                                        op0=ALU.min)
                st = wk3.tile([128, 128], F32, name="st")
                nc.vector.tensor_tensor(st[:], sta[:], stb[:],
                                        op=ALU.subtract)
                vg = wk3.tile([128, 256], BF16, name="vg")
                nc.gpsimd.indirect_dma_start(
                    out=vg[:], out_offset=None, in_=vtab_full[:],
                    in_offset=bass.IndirectOffsetOnAxis(ap=ert[:, cc:cc + 1],
                                                        axis=0))
                vgf = wk3.tile([128, 256], F32, name="vgf")
                nc.vector.tensor_copy(vgf[:], vg[:])
                nc.tensor.matmul(pc[:], lhsT=st[:], rhs=vgf[:],
                                 start=(cc == 0), stop=(cc == cw[c] - 1))
            rsib = wk3.tile([128, 1], BF16, name="rsib")
            if rows < 128:
                nc.gpsimd.memset(rsib[:], 0.0)
            nc.sync.dma_start(rsib[0:rows, :],
                              io["rsio"][c * 128:c * 128 + rows, 0:1])
            rsi = wk3.tile([128, 1], F32, name="rsi")
            nc.vector.tensor_copy(rsi[:], rsib[:])
            x2 = wk3.tile([128, 256], F32, name="x2")
            nc.vector.tensor_scalar(x2[:], pc[:], rsi[:, 0:1], None,
                                    op0=ALU.mult)
            nc.vector.tensor_tensor(x2[:], x2[:], sigb, op=ALU.mult)
            nc.vector.tensor_add(xt[:], xt[:], x2[:])
            # ---- output projection
            tp0 = ps_tp.tile([128, 128], F32, name="tp0")
            nc.tensor.transpose(tp0[:], xt[:, 0:128], ident[:])
            tp1 = ps_tp.tile([128, 128], F32, name="tp1")
            nc.tensor.transpose(tp1[:], xt[:, 128:256], ident[:])
            xt0 = wk3.tile([128, 128], F32, name="xt0")
            nc.vector.tensor_copy(xt0[:], tp0[:])
            xt1 = wk3.tile([128, 128], F32, name="xt1")
            nc.vector.tensor_copy(xt1[:], tp1[:])
            po = ps_out.tile([128, 64], F32, name="po")
            nc.tensor.matmul(po[:], lhsT=xt0[:], rhs=woT0[:],
                             start=True, stop=False)
            nc.tensor.matmul(po[:], lhsT=xt1[:], rhs=woT1[:],
                             start=False, stop=True)
            osb = wk3.tile([128, 64], F32, name="osb")
            nc.vector.tensor_tensor(osb[:], po[:], woscl, op=ALU.mult)
            nc.vector.tensor_add(osb[:], osb[:], wob)
            # int8 output quantization with per-row (node) scale
            aabs = wk3.tile([128, 64], F32, name="aabs")
            nc.scalar.activation(aabs[:], osb[:], ACT.Abs)
            am = wk3.tile([128, 1], F32, name="am")
            nc.vector.tensor_reduce(am[:], aabs[:], axis=AX.X, op=ALU.max)
            nc.vector.tensor_scalar(am[:], am[:], 1e-30, None, op0=ALU.add)
            # round the scale to bf16 FIRST so quant and host dequant agree
            amb = wk3.tile([128, 1], BF16, name="amb")
            nc.vector.tensor_copy(amb[:], am[:])
            am2 = wk3.tile([128, 1], F32, name="am2")
            nc.vector.tensor_copy(am2[:], amb[:])
            rec = wk3.tile([128, 1], F32, name="orec")
            nc.vector.reciprocal(rec[:], am2[:])
            nc.vector.tensor_scalar(rec[:], rec[:], 127.0, None, op0=ALU.mult)
            oq = wk3.tile([128, 64], F32, name="oq")
            nc.vector.tensor_scalar(oq[:], osb[:], rec[:, 0:1], None,
                                    op0=ALU.mult)
            ob = wk3.tile([128, 64], I8, name="ob")
            nc.vector.tensor_copy(ob[:], oq[:])
            nc.sync.dma_start(
                out_d[c * 128:c * 128 + rows, 0:64].bitcast(I8), ob[0:rows, :])
            nc.sync.dma_start(
                out_d[c * 128:c * 128 + rows, 64:66].bitcast(BF16),
                amb[0:rows, :])


# ------------------------------------------------------------- cached runner
class _Runner:
    """One compiled jit(shard_map(bass_exec)) executable, reused per call."""

    def __init__(self, cw, off, cwt):
        import jax
        from jax.sharding import Mesh, PartitionSpec, NamedSharding
        from jax.experimental.shard_map import shard_map
        from concourse.bass2jax import (_bass_exec_p, install_neuronx_cc_hook,
                                        partition_id_tensor)

        install_neuronx_cc_hook()
        try:
            # strip source paths from HLO metadata so the PJRT NEFF cache
            # key is independent of where kernel.py lives
            jax.config.update("jax_hlo_source_file_canonicalization_regex",
                              ".*")
        except Exception:
            pass
        nc = bacc.Bacc("TRN2", target_bir_lowering=False, debug=False,
                       enable_asserts=False, num_devices=NCORE)
        with tile.TileContext(nc) as tc:
            with ExitStack() as ctx:
                _build(nc, tc, ctx, cw, off, cwt)
        nc.compile()
        self.nc = nc

        partition_name = nc.partition_id_tensor.name if nc.partition_id_tensor else None
        in_names, out_names, out_avals = [], [], []
        for alloc in nc.m.functions[0].allocations:
            if not isinstance(alloc, mybir.MemoryLocationSet):
                continue
            name = alloc.memorylocations[0].name
            if alloc.kind == "ExternalInput":
                if name != partition_name:
                    in_names.append(name)
            elif alloc.kind == "ExternalOutput":
                out_avals.append(jax.core.ShapedArray(
                    tuple(alloc.tensor_shape), mybir.dt.np(alloc.dtype)))
                out_names.append(name)
        in_names_full = in_names + out_names + (
            [partition_name] if partition_name else [])
        self.in_names = in_names
        self.out_names = out_names

        def _body(*args):
            operands = list(args)
            if partition_name is not None:
                operands.append(partition_id_tensor())
            return tuple(_bass_exec_p.bind(
                *operands, out_avals=tuple(out_avals),
                in_names=tuple(in_names_full), out_names=tuple(out_names),
                lowering_input_output_aliases=(),
                sim_require_finite=True, sim_require_nnan=True, nc=nc))

        devices = jax.devices()[:NCORE]
        mesh = Mesh(np.asarray(devices), ("core",))
        nargs = len(in_names) + len(out_names)
        self._jitted = jax.jit(
            shard_map(_body, mesh=mesh,
                      in_specs=(PartitionSpec("core"),) * nargs,
                      out_specs=(PartitionSpec("core"),) * len(out_names),
                      check_rep=False),
            keep_unused=True)
        sh = NamedSharding(mesh, PartitionSpec("core"))
        self._zero_args = [
            jax.device_put(
                np.zeros((NCORE * a.shape[0], *a.shape[1:]), a.dtype), sh)
            for a in out_avals]
        for z in self._zero_args:
            z.block_until_ready()
        from concurrent.futures import ThreadPoolExecutor
        self._pool = ThreadPoolExecutor(18)

    def _fetch(self, out) -> np.ndarray:
        """Parallel per-shard device->host fetch, reassembled in core order."""
        shards = sorted(out.addressable_shards,
                        key=lambda s: (s.index[0].start or 0))
        parts = list(self._pool.map(lambda s: np.asarray(s.data), shards))
        return np.concatenate(parts, axis=0)

    def run(self, arrs) -> np.ndarray:
        """One retry: the axon tunnel can throw transient INTERNAL errors on
        fetch under sustained load; a clean second attempt recovers."""
        try:
            return self._run_once(arrs)
        except Exception:
            return self._run_once(arrs)

    def _run_once(self, arrs) -> np.ndarray:
        """arrs: name -> global concatenated array. Returns full [B,N,64] f32.

        Each fetch thread dequantizes its own shard as soon as its bytes
        arrive, overlapping host work with the other shards' transfers."""
        args = [arrs[nm] for nm in self.in_names]
        outs = self._jitted(*args, *self._zero_args)
        shards = sorted(outs[0].addressable_shards,
                        key=lambda s: (s.index[0].start or 0))
        res = np.empty((N, 64), np.float32)

        def work(item):
            i, shd = item
            buf = np.asarray(shd.data)                     # [NSH, 66] u8
            q = buf[:, :64].view(np.int8)
            s = np.ascontiguousarray(buf[:, 64:66]).view(bfloat16)
            s = s.astype(np.float32)
            np.multiply(q.astype(np.float32), s * (1.0 / 127.0),
                        out=res[i * NSH:(i + 1) * NSH])

        list(self._pool.map(work, enumerate(shards)))
        return res.reshape(B, N, 64)


_CACHE = {}


def _get_runner(cw, off, cwt):
    key = (cwt, tuple(cw))
    if key not in _CACHE:
        _CACHE[key] = _Runner(cw, off, cwt)
    return _CACHE[key]


def kernel(**inputs) -> np.ndarray:
    inputs = {k: np.asarray(v) for k, v in inputs.items()}
    arrs, cw, off, cwt = _prep(**inputs)
    runner = _get_runner(cw, off, cwt)
    return runner.run(arrs)
